# revision 50
# baseline (speedup 1.0000x reference)
"""Trainium2 Bass kernel for GemNet AtomUpdateBlock (gnn_message_passing).

Computation (per reference):
    bases = basis_rad @ W_rbf              # [E, De]
    x     = m * bases                      # [E, De]
    z     = segment_sum(x, idx_atom, A)    # [A, De]
    x     = silu(z @ W_in)                 # [A, Da]
    3x residual: x = (x + silu(silu(x W1) W2)) / sqrt(2)

Distribution strategy: shard EDGES BY DESTINATION ATOM. The host bins the
atoms into 8 cores x T_ATOM tiles of <=128 atoms (balanced by edge count),
sorts/pads each tile's edges into K 128-edge groups, and each core computes
the segment-sum + atom MLP for its own atoms only. No collective needed;
outputs are disjoint atom slices.

Per 128-edge column on device (bf16 matmuls, f32 PSUM):
    PE:  bases = basis_colT.T @ W_rbf  -- TWO columns packed per PE pass via
         row-tiling (tile_position (0,0)/(32,0), contraction K=16 each,
         concurrent sub-array execution), f32 into two PSUM banks.
    x = bases * m: the PSUM-sourced multiply is the DVE-1x wall (f32 PSUM
         read port), so pairs alternate between two routes to balance the
         engines: direct DVE tensor_tensor (PSUM f32 x SBUF bf16), or
         ACT-engine evac to bf16 SBUF followed by a 2x-packed all-bf16 DVE
         multiply.
    PE:  TRANSPOSED scatter: zT[d,a] += x[:,dchunk].T @ S per 128-d-chunk
         (lhsT = x slice, rhs = one-hot S column). Produces z already
         feature-major, so the epilogue needs NO PE transposes and no
         z evac chain -- one fused ACT copy per subtile. The four d-chunk
         accumulators share one PSUM bank via interleaved accumulation
         (start=True only on the very first matmul of the subtile: the
         bank-wide has_written clear makes the other chunks' first writes
         overwrite-then-accumulate).
The (bases-pair, mult, scatter) stream is software-pipelined, edge-stream
DMAs are prefetched three subtiles ahead, and the previous quad's epilogue
matmuls are interleaved into the scatter stream (keeps the PE HAM clock
gate at 8/8). A warmup matmul burst upclocks the PE while the first DMAs
stream in; weight/const DMAs and output writes ride the scalar engine's
DMA queue so the sync-queue edge stream is never delayed.
Epilogue per QUAD of 128-atom tiles (512 atoms, feature-major): bf16 MLP
matmuls at N=512, silu on ACT, skip-adds as one fused DVE
scalar_tensor_tensor per layer with host-folded sqrt2 scaling. Output is
written feature-major [P, Cj*T*P] bf16 and untransposed/cast on the host
during unshard.
"""

import math
import os
import sys

import numpy as np
import ml_dtypes

BF16 = ml_dtypes.bfloat16

P = 128
N_CORES = 8
DE, DA, DR, NH = 512, 256, 16, 3
T_ATOM = 20  # atom tiles per core (each up to 128 atoms); divisible by 4
INV_SQRT_2 = 0.7071067811865476

# Of every EVAC_DEN bases-pairs, EVAC_NUM are routed via ACT-evac + 2x DVE;
# the rest are multiplied directly from PSUM at DVE 1x. Balances ACT vs DVE.
EVAC_NUM, EVAC_DEN = 4, 7
PREFETCH = 3  # subtiles of DMA lookahead

_NC_CACHE = {}


# ----------------------------------------------------------------------------
# Host-side packing
# ----------------------------------------------------------------------------

def _pack_layout(idx, n_atoms, n_cores, t_atom):
    E = idx.shape[0]
    n_bins = n_cores * t_atom
    counts = np.bincount(idx, minlength=n_atoms)

    order = np.argsort(-counts, kind="stable")
    n_rounds = math.ceil(n_atoms / n_bins)
    pad = n_rounds * n_bins - n_atoms
    padded = np.concatenate([order, np.full(pad, -1, dtype=order.dtype)])
    grid = padded.reshape(n_rounds, n_bins)
    grid[1::2] = grid[1::2, ::-1]  # snake-deal: balances edges and atoms
    bin_of_atom = np.empty(n_atoms, dtype=np.int64)
    slot_of_atom = np.empty(n_atoms, dtype=np.int64)
    valid = grid >= 0
    bin_idx = np.broadcast_to(np.arange(n_bins), grid.shape)
    round_idx = np.broadcast_to(np.arange(n_rounds)[:, None], grid.shape)
    bin_of_atom[grid[valid]] = bin_idx[valid]
    slot_of_atom[grid[valid]] = round_idx[valid]
    assert np.bincount(bin_of_atom, minlength=n_bins).max() <= P

    ebin = bin_of_atom[idx]
    eslot = slot_of_atom[idx]
    eorder = np.argsort(ebin * (P + 1) + eslot, kind="stable")
    ebin_sorted = ebin[eorder]
    bin_counts = np.bincount(ebin_sorted, minlength=n_bins)
    K = max(1, math.ceil(bin_counts.max() / P))
    bin_starts = np.zeros(n_bins + 1, dtype=np.int64)
    np.cumsum(bin_counts, out=bin_starts[1:])
    pos_in_bin = np.arange(E) - bin_starts[ebin_sorted]

    core_of_bin = np.arange(n_bins) // t_atom
    tile_of_bin = np.arange(n_bins) % t_atom
    return dict(
        K=K,
        eorder=eorder,
        core_of_edge=core_of_bin[ebin_sorted],
        flat_slot=tile_of_bin[ebin_sorted] * (K * P) + pos_in_bin,
        rel_of_edge=eslot[eorder].astype(np.int64),
        bin_of_atom=bin_of_atom,
        slot_of_atom=slot_of_atom,
        core_of_bin=core_of_bin,
        tile_of_bin=tile_of_bin,
    )


def _pack_weights(W_rbf, W_in, res_W1, res_W2):
    Ci, Cj = DE // P, DA // P
    Cr = DA // P
    # wrbf duplicated at partition rows 0:16 and 32:48 so two row-tiled
    # bases matmuls (tile_position (0,0)/(32,0)) share one rhs tile.
    wrbf2 = np.zeros((64, DE), dtype=np.float32)
    wrbf2[0:DR] = W_rbf
    wrbf2[32 : 32 + DR] = W_rbf
    win = W_in.reshape(Ci, P, Cj, P).transpose(1, 0, 2, 3).reshape(P, Ci * Cj * P)
    blocks = []
    c = INV_SQRT_2
    for l in range(NH):
        w1 = (res_W1[l] * (c ** l)).astype(np.float32)
        w2 = res_W2[l].astype(np.float32)
        for W in (w1, w2):
            blocks.append(
                W.reshape(Cr, P, Cr, P).transpose(1, 0, 2, 3).reshape(P, Cr * Cr * P)
            )
    wres = np.concatenate(blocks, axis=1)
    return (
        np.ascontiguousarray(wrbf2, dtype=BF16),
        np.ascontiguousarray(win, dtype=BF16),
        np.ascontiguousarray(wres, dtype=BF16),
    )


def _build_in_maps(m, basis_rad, layout, W_rbf, W_in, res_W1, res_W2, n_cores, t_atom):
    K = layout["K"]
    P_K = (K + 1) // 2
    cap = t_atom * K * P
    ncols = t_atom * K
    eorder = layout["eorder"]
    core_of_edge = layout["core_of_edge"]
    flat_slot = layout["flat_slot"]
    rel = layout["rel_of_edge"]

    wrbf2, win, wres = _pack_weights(W_rbf, W_in, res_W1, res_W2)
    m_src = m[eorder]
    bas_src = basis_rad[eorder]

    in_maps = []
    for c in range(n_cores):
        sel = core_of_edge == c
        fs = flat_slot[sel]
        m_pack = np.zeros((cap, DE), dtype=BF16)
        m_pack[fs] = m_src[sel].astype(BF16)
        # partition-major: m2[p, col*DE + d] = m_pack[col*P + p, d] so each
        # partition's per-atom-tile DMA read is fully contiguous
        m_pack = np.ascontiguousarray(
            m_pack.reshape(ncols, P, DE).transpose(1, 0, 2).reshape(P, ncols, DE)
        )
        basT = np.zeros((DR, cap), dtype=np.float32)
        basT[:, fs] = bas_src[sel].T
        # pair-packed basis, dense 32 rows: rows 0:16 hold even columns,
        # rows 16:32 odd columns; DMA'd as two 16-row slices into SBUF
        # partitions 0:16 and 32:48 for the row-tiled 2-pack bases matmul.
        bt = basT.reshape(DR, t_atom, K, P)
        b2 = np.zeros((32, t_atom, P_K, P), dtype=np.float32)
        b2[0:DR] = bt[:, :, 0::2, :]
        n_odd = K // 2
        if n_odd:
            b2[16 : 16 + DR, :, 0:n_odd, :] = bt[:, :, 1::2, :]
        basT2 = np.ascontiguousarray(
            b2.reshape(32, t_atom * P_K * P), dtype=BF16
        )
        rel_flat = np.full(cap, -1, dtype=np.int64)
        rel_flat[fs] = rel[sel]
        rel2 = rel_flat.reshape(ncols, P).T  # [p, col]
        s_host = (rel2[:, :, None] == np.arange(P)[None, None, :]).astype(
            ml_dtypes.float8_e4m3
        )
        in_maps.append(
            dict(
                m_pack=m_pack,
                basT2=basT2,
                s_hot=np.ascontiguousarray(s_host.reshape(P, ncols * P)),
                wrbf2=wrbf2,
                win=win,
                wres=wres,
            )
        )
    return in_maps


def _unpack_output(results, layout, n_atoms, n_cores, t_atom):
    Cj = DA // P
    out = np.zeros((n_atoms, DA), dtype=np.float32)
    core_of_atom = layout["core_of_bin"][layout["bin_of_atom"]]
    row_of_atom = (
        layout["tile_of_bin"][layout["bin_of_atom"]] * P + layout["slot_of_atom"]
    )
    for c in range(n_cores):
        x = results[c]["out"].astype(np.float32).reshape(P, Cj, t_atom, P)
        x_core = x.transpose(2, 3, 1, 0).reshape(t_atom * P, DA)
        mask = core_of_atom == c
        out[mask] = x_core[row_of_atom[mask]]
    return out


# ----------------------------------------------------------------------------
# Bass kernel builder
# ----------------------------------------------------------------------------

def _build_nc(t_atom, K):
    import concourse.mybir as mybir
    import concourse.tile as tile
    from concourse import bacc

    f32 = mybir.dt.float32
    bf16 = mybir.dt.bfloat16
    Ci, Cj = DE // P, DA // P
    Cr = DA // P
    P_K = (K + 1) // 2
    ncols = t_atom * K
    C3 = INV_SQRT_2 ** NH
    GAMMA = [float((1.0 / INV_SQRT_2) ** l) for l in range(NH)]
    assert t_atom % 4 == 0
    n_quads = t_atom // 4
    W4 = 4 * P  # atoms per epilogue quad

    nc = bacc.Bacc(
        "TRN2",
        target_bir_lowering=False,
        debug=False,
        enable_asserts=False,
        num_devices=N_CORES,
    )
    d_m = nc.dram_tensor("m_pack", [P, ncols, DE], bf16, kind="ExternalInput")
    d_basT2 = nc.dram_tensor(
        "basT2", [32, t_atom * P_K * P], bf16, kind="ExternalInput"
    )
    f8 = mybir.dt.float8e4
    d_s = nc.dram_tensor("s_hot", [P, ncols * P], f8, kind="ExternalInput")
    d_wrbf2 = nc.dram_tensor("wrbf2", [64, DE], bf16, kind="ExternalInput")
    d_win = nc.dram_tensor("win", [P, Ci * Cj * P], bf16, kind="ExternalInput")
    d_wres = nc.dram_tensor(
        "wres", [P, NH * 2 * Cr * Cr * P], bf16, kind="ExternalInput"
    )
    d_out = nc.dram_tensor("out", [P, Cj * t_atom * P], bf16, kind="ExternalOutput")

    with tile.TileContext(nc) as tc:
        with (
            tc.tile_pool(name="const", bufs=1) as const_p,
            tc.tile_pool(name="bas", bufs=6) as bas_p,
            tc.tile_pool(name="m", bufs=6) as m_p,
            tc.tile_pool(name="s", bufs=6) as s_p,
            tc.tile_pool(name="x", bufs=6) as x_p,
            tc.tile_pool(name="bb", bufs=3) as bb_p,
            tc.tile_pool(name="ztsb", bufs=2) as ztsb_p,
            tc.tile_pool(name="act", bufs=3) as act_p,
            tc.tile_pool(name="outp", bufs=2) as out_p,
            tc.tile_pool(name="psb", bufs=2, space="PSUM") as psb_p,
            tc.tile_pool(name="psz", bufs=2, space="PSUM") as psz_p,
            tc.tile_pool(name="psm", bufs=2, space="PSUM") as psm_p,
        ):
            def emit_silu(out_ap, in_ps_ap):
                nc.scalar.activation(
                    out=out_ap, in_=in_ps_ap,
                    func=mybir.ActivationFunctionType.Silu,
                )

            # wrbf2 first on the sync queue so the edge stream follows
            # immediately; the other consts ride the scalar engine's DMA
            # queue so they never delay the edge-stream DMAs.
            wrbf_sb = const_p.tile([64, DE], bf16, tag="wrbf")
            nc.sync.dma_start(out=wrbf_sb[:], in_=d_wrbf2[:])
            win_sb = const_p.tile([P, Ci * Cj * P], bf16, tag="win")
            nc.scalar.dma_start(out=win_sb[:], in_=d_win[:])
            wres_sb = const_p.tile([P, NH * 2 * Cr * Cr * P], bf16, tag="wres")
            nc.scalar.dma_start(out=wres_sb[:], in_=d_wres[:])

            # HAM warmup: dense back-to-back full-K matmuls upclock the PE
            # (4/8 -> 8/8) while the first edge DMAs land. The operand tile
            # is memset on GpSimd (not DMA'd) so the burst starts right
            # after the framework preamble instead of waiting on HBM.
            warm_in = const_p.tile([P, DE], bf16, tag="warmin")
            nc.gpsimd.memset(warm_in[:], 0.0)
            warm_ps = psb_p.tile([P, 2, DE], f32, space="PSUM", tag="bases",
                                 name="warm")
            for w in range(16):
                nc.tensor.matmul(
                    out=warm_ps[:, w % 2, :],
                    lhsT=warm_in[:, (w % 4) * P : (w % 4 + 1) * P],
                    rhs=warm_in[:],
                    start=True,
                    stop=True,
                )

            def issue_sub_dmas(t):
                bas_sb = bas_p.tile([48, P_K * P], bf16, tag="bas",
                                    name=f"bas{t}")
                nc.sync.dma_start(
                    out=bas_sb[0:16, :],
                    in_=d_basT2[0:16, t * P_K * P : (t + 1) * P_K * P],
                )
                nc.sync.dma_start(
                    out=bas_sb[32:48, :],
                    in_=d_basT2[16:32, t * P_K * P : (t + 1) * P_K * P],
                )
                m_t = m_p.tile([P, K, DE], bf16, tag="m", name=f"m{t}")
                s_t = s_p.tile([P, K * P], f8, tag="s", name=f"s{t}")
                if t == 0:
                    # split the very first m transfer (and land s before its
                    # tail) so the leading columns start ~3us earlier
                    h = max(1, K // 3)
                    nc.sync.dma_start(
                        out=m_t[:, 0:h, :], in_=d_m[:, 0:h, :]
                    )
                    nc.sync.dma_start(
                        out=s_t[:], in_=d_s[:, t * K * P : (t + 1) * K * P]
                    )
                    nc.sync.dma_start(
                        out=m_t[:, h:K, :], in_=d_m[:, h:K, :]
                    )
                else:
                    nc.sync.dma_start(
                        out=m_t[:], in_=d_m[:, t * K : (t + 1) * K, :]
                    )
                    nc.sync.dma_start(
                        out=s_t[:], in_=d_s[:, t * K * P : (t + 1) * K * P]
                    )
                return (bas_sb, m_t, s_t)

            def epilogue_gen(q, zt_sb, fill=False):
                """Quad epilogue emitted as units interleavable with the next
                quad's scatter stream (keeps PE continuously busy for HAM).
                With fill=True (the final quad, which drains with no scatter
                stream to interleave into), latency boundaries are padded
                with dummy full-K matmuls so the PE holds its 8/8 clock and
                never sits idle waiting on ACT/DVE chains."""
                fill_t = [None]

                def pad(n):
                    if not fill:
                        return
                    if fill_t[0] is None:
                        fill_t[0] = psb_p.tile(
                            [P, 2, DE], f32, space="PSUM", tag="bases",
                            name="tailfill"
                        )
                    for w in range(n):
                        nc.tensor.matmul(
                            out=fill_t[0][:, 0, :],
                            lhsT=warm_in[:, (w % 4) * P : (w % 4 + 1) * P],
                            rhs=warm_in[:],
                            start=True,
                            stop=True,
                        )

                u_ps = [
                    psm_p.tile(
                        [P, W4], f32, space="PSUM", tag="misc", name=f"ups{q}_{j}"
                    )
                    for j in range(Cj)
                ]
                # sub-major N=128 matmuls with interleaved per-region
                # accumulation: the first stepped units only need subtile 0's
                # evac, so the epilogue never waits on the last subtile.
                for sub in range(4):
                    for c in range(Ci):
                        for j in range(Cj):
                            fi = c * Cj + j
                            nc.tensor.matmul(
                                out=u_ps[j][:, sub * P : (sub + 1) * P],
                                lhsT=win_sb[:, fi * P : (fi + 1) * P],
                                rhs=zt_sb[:, c, sub, :],
                                start=(sub == 0 and c == 0),
                                stop=(sub == 3 and c == Ci - 1),
                                skip_group_check=True,
                            )
                        yield
                X = act_p.tile([P, Cr * W4], bf16, tag="X", name=f"X{q}_0")
                for j in range(Cj):
                    emit_silu(X[:, j * W4 : (j + 1) * W4], u_ps[j][:])
                pad(4)
                yield
                for l in range(NH):
                    v_ps = [
                        psm_p.tile(
                            [P, W4], f32, space="PSUM", tag="misc",
                            name=f"vps{q}_{l}_{j}"
                        )
                        for j in range(Cr)
                    ]
                    for j in range(Cr):
                        for i in range(Cr):
                            fi = ((l * 2 + 0) * Cr + i) * Cr + j
                            nc.tensor.matmul(
                                out=v_ps[j][:],
                                lhsT=wres_sb[:, fi * P : (fi + 1) * P],
                                rhs=X[:, i * W4 : (i + 1) * W4],
                                start=(i == 0),
                                stop=(i == Cr - 1),
                            )
                            yield
                    u1 = act_p.tile([P, Cr * W4], bf16, tag="u1", name=f"u1_{q}_{l}")
                    for j in range(Cr):
                        emit_silu(u1[:, j * W4 : (j + 1) * W4], v_ps[j][:])
                    pad(3)
                    yield
                    w_ps = [
                        psm_p.tile(
                            [P, W4], f32, space="PSUM", tag="misc",
                            name=f"wps{q}_{l}_{j}"
                        )
                        for j in range(Cr)
                    ]
                    for j in range(Cr):
                        for i in range(Cr):
                            fi = ((l * 2 + 1) * Cr + i) * Cr + j
                            nc.tensor.matmul(
                                out=w_ps[j][:],
                                lhsT=wres_sb[:, fi * P : (fi + 1) * P],
                                rhs=u1[:, i * W4 : (i + 1) * W4],
                                start=(i == 0),
                                stop=(i == Cr - 1),
                            )
                            yield
                    Y = act_p.tile([P, Cr * W4], bf16, tag="y", name=f"Y{q}_{l}")
                    for j in range(Cr):
                        emit_silu(Y[:, j * W4 : (j + 1) * W4], w_ps[j][:])
                    pad(3)
                    yield
                    Xn = act_p.tile(
                        [P, Cr * W4], bf16, tag="X", name=f"X{q}_{l + 1}"
                    )
                    nc.vector.scalar_tensor_tensor(
                        out=Xn[:],
                        in0=Y[:],
                        scalar=GAMMA[l],
                        in1=X[:],
                        op0=mybir.AluOpType.mult,
                        op1=mybir.AluOpType.add,
                    )
                    X = Xn
                    pad(4)
                    yield
                o_t = out_p.tile([P, Cj * W4], bf16, tag="out")
                nc.vector.tensor_scalar(
                    out=o_t[:], in0=X[:], scalar1=float(C3), scalar2=None,
                    op0=mybir.AluOpType.mult,
                )
                for j in range(Cj):
                    nc.scalar.dma_start(
                        out=d_out[
                            :, (j * t_atom + 4 * q) * P : (j * t_atom + 4 * q + 4) * P
                        ],
                        in_=o_t[:, j * W4 : (j + 1) * W4],
                    )
                yield

            tiles = {}
            for t in range(min(PREFETCH, t_atom)):
                tiles[t] = issue_sub_dmas(t)

            prev_epi = None
            pair_ctr = [0]
            dmy_state = [None, 0]
            for q in range(n_quads):
                zt_sb = ztsb_p.tile([P, Ci, 4, P], bf16, tag="ztsb")

                def step_epi():
                    if prev_epi is not None:
                        next(prev_epi, None)
                    else:
                        # quad 0 has no epilogue to interleave; burn a full-K
                        # matmul on three of every five steps so HAM sees
                        # enough PE activity to hold the 8/8 clock.
                        w = dmy_state[1]
                        dmy_state[1] += 1
                        if w % 5 in (1, 3):
                            return
                        if dmy_state[0] is None:
                            dmy_state[0] = psm_p.tile(
                                [P, W4], f32, space="PSUM", tag="misc",
                                name="dmy"
                            )
                        nc.tensor.matmul(
                            out=dmy_state[0][:],
                            lhsT=warm_in[:, (w % 4) * P : (w % 4 + 1) * P],
                            rhs=warm_in[:],
                            start=True,
                            stop=True,
                        )

                for sub in range(4):
                    t = 4 * q + sub
                    tf = t + PREFETCH
                    if tf < t_atom:
                        tiles[tf] = issue_sub_dmas(tf)
                    bas_sb, m_t, s_t = tiles.pop(t)
                    zt_ps = psz_p.tile(
                        [P, Ci, P], f32, space="PSUM", tag="z", name=f"ztps{t}"
                    )

                    def emit_mult(psb_t, p, evac, m_t=m_t, t=t):
                        """DVE/ACT multiply, emitted right at pair issue so the
                        mult chain runs several pairs ahead of the PE's
                        scatter consumption."""
                        c0 = 2 * p
                        w = 2 if c0 + 1 < K else 1
                        x_t = x_p.tile([P, 2, DE], bf16, tag="x",
                                       name=f"x{t}_{p}")
                        if evac:
                            bb = bb_p.tile([P, 2, DE], bf16, tag="bb",
                                           name=f"bb{t}_{p}")
                            nc.scalar.copy(
                                out=bb[:, 0:w, :], in_=psb_t[:, 0:w, :]
                            )
                            nc.vector.tensor_tensor(
                                out=x_t[:, 0:w, :],
                                in0=bb[:, 0:w, :],
                                in1=m_t[:, c0 : c0 + w, :],
                                op=mybir.AluOpType.mult,
                            )
                        else:
                            nc.vector.tensor_tensor(
                                out=x_t[:, 0:w, :],
                                in0=psb_t[:, 0:w, :],
                                in1=m_t[:, c0 : c0 + w, :],
                                op=mybir.AluOpType.mult,
                            )
                        return x_t

                    def emit_scatter(item, s_t=s_t, zt_ps=zt_ps):
                        x_t, p = item
                        c0 = 2 * p
                        w = 2 if c0 + 1 < K else 1
                        for i in range(w):
                            c = c0 + i
                            for ci in range(Ci):
                                nc.tensor.matmul(
                                    out=zt_ps[:, ci, :],
                                    lhsT=x_t[:, i, ci * P : (ci + 1) * P],
                                    rhs=s_t[:, c * P : (c + 1) * P],
                                    start=(c == 0 and ci == 0),
                                    stop=(c == K - 1 and ci == Ci - 1),
                                    skip_group_check=True,
                                )

                    # pairs are emitted in blocks of two: consecutive pairs
                    # reuse the same row-groups back-to-back, so the ~100ns
                    # row-tile -> full-row handoff is paid once per block
                    # instead of once per pair.
                    pend = []
                    for pb in range(0, P_K, 2):
                        blk = [p for p in (pb, pb + 1) if p < P_K]
                        xs = []
                        for p in blk:
                            c0 = 2 * p
                            psb_t = psb_p.tile(
                                [P, 2, DE], f32, space="PSUM", tag="bases",
                                name=f"bps{t}_{p}"
                            )
                            nc.tensor.matmul(
                                out=psb_t[:, 0, :],
                                lhsT=bas_sb[0:DR, p * P : (p + 1) * P],
                                rhs=wrbf_sb[0:DR, :],
                                start=True,
                                stop=True,
                                tile_position=(0, 0),
                            )
                            if c0 + 1 < K:
                                nc.tensor.matmul(
                                    out=psb_t[:, 1, :],
                                    lhsT=bas_sb[32 : 32 + DR, p * P : (p + 1) * P],
                                    rhs=wrbf_sb[32 : 32 + DR, :],
                                    start=True,
                                    stop=True,
                                    tile_position=(32, 0),
                                )
                            xs.append((psb_t, p))
                        for psb_t, p in xs:
                            evac = (pair_ctr[0] * EVAC_NUM) % EVAC_DEN < EVAC_NUM
                            pair_ctr[0] += 1
                            pend.append((emit_mult(psb_t, p, evac), p))
                        # epilogue steps between the bases pair and the
                        # scatter burst: a full-row N=512 epilogue matmul
                        # absorbs the row-tile -> full-row drain handoff that
                        # a short N=128 scatter matmul would otherwise pay.
                        for _ in range(2 * len(blk)):
                            step_epi()
                        while len(pend) > 2:
                            emit_scatter(pend.pop(0))
                    for item in pend:
                        emit_scatter(item)

                    # single fused evac: zT chunks -> bf16 zt_sb, no PE work
                    nc.scalar.copy(out=zt_sb[:, :, sub, :], in_=zt_ps[:])
                    step_epi()

                if prev_epi is not None:
                    for _ in prev_epi:
                        pass
                prev_epi = epilogue_gen(q, zt_sb, fill=(q == n_quads - 1))
            for _ in prev_epi:
                pass

    nc.compile()
    return nc


def _get_nc(t_atom, K):
    key = (t_atom, K)
    if key not in _NC_CACHE:
        _NC_CACHE[key] = _build_nc(t_atom, K)
    return _NC_CACHE[key]


# ----------------------------------------------------------------------------
# Entry point
# ----------------------------------------------------------------------------

def kernel(h, m, basis_rad, idx_atom, W_rbf, W_in, res_W1, res_W2):
    from concourse.bass_utils import run_bass_kernel_spmd

    m = np.asarray(m, dtype=np.float32)
    basis_rad = np.asarray(basis_rad, dtype=np.float32)
    idx = np.asarray(idx_atom).astype(np.int64)
    W_rbf = np.asarray(W_rbf, dtype=np.float32)
    W_in = np.asarray(W_in, dtype=np.float32)
    res_W1 = np.asarray(res_W1, dtype=np.float32)
    res_W2 = np.asarray(res_W2, dtype=np.float32)
    n_atoms = np.asarray(h).shape[0]

    layout = _pack_layout(idx, n_atoms, N_CORES, T_ATOM)
    in_maps = _build_in_maps(
        m, basis_rad, layout, W_rbf, W_in, res_W1, res_W2, N_CORES, T_ATOM
    )
    nc = _get_nc(T_ATOM, layout["K"])

    trace = os.environ.get("KERNEL_TRACE", "0") == "1"
    res = run_bass_kernel_spmd(
        nc, in_maps, core_ids=list(range(N_CORES)), trace=trace
    )
    if trace and res.exec_time_ns is not None:
        print(f"HW exec time: {res.exec_time_ns} ns", file=sys.stderr)
        kernel.last_exec_time_ns = res.exec_time_ns
    kernel.last_results = res
    return _unpack_output(res.results, layout, n_atoms, N_CORES, T_ATOM)


# revision 51
# speedup vs baseline: 1.0196x; 1.0196x over previous
"""Trainium2 Bass kernel for GemNet AtomUpdateBlock (gnn_message_passing).

Computation (per reference):
    bases = basis_rad @ W_rbf              # [E, De]
    x     = m * bases                      # [E, De]
    z     = segment_sum(x, idx_atom, A)    # [A, De]
    x     = silu(z @ W_in)                 # [A, Da]
    3x residual: x = (x + silu(silu(x W1) W2)) / sqrt(2)

Distribution strategy: shard EDGES BY DESTINATION ATOM. The host bins the
atoms into 8 cores x T_ATOM tiles of <=128 atoms (balanced by edge count),
sorts/pads each tile's edges into K 128-edge groups, and each core computes
the segment-sum + atom MLP for its own atoms only. No collective needed;
outputs are disjoint atom slices.

Per 128-edge column on device (bf16 matmuls, f32 PSUM):
    PE:  bases = basis_colT.T @ W_rbf  -- TWO columns packed per PE pass via
         row-tiling (tile_position (0,0)/(32,0), contraction K=16 each,
         concurrent sub-array execution), f32 into two PSUM banks.
    x = bases * m: the PSUM-sourced multiply is the DVE-1x wall (f32 PSUM
         read port), so pairs alternate between two routes to balance the
         engines: direct DVE tensor_tensor (PSUM f32 x SBUF bf16), or
         ACT-engine evac to bf16 SBUF followed by a 2x-packed all-bf16 DVE
         multiply.
    PE:  TRANSPOSED scatter: zT[d,a] += x[:,dchunk].T @ S per 128-d-chunk
         (lhsT = x slice, rhs = one-hot S column). Produces z already
         feature-major, so the epilogue needs NO PE transposes and no
         z evac chain -- one fused ACT copy per subtile. The four d-chunk
         accumulators share one PSUM bank via interleaved accumulation
         (start=True only on the very first matmul of the subtile: the
         bank-wide has_written clear makes the other chunks' first writes
         overwrite-then-accumulate).
The (bases-pair, mult, scatter) stream is software-pipelined, edge-stream
DMAs are prefetched three subtiles ahead, and the previous quad's epilogue
matmuls are interleaved into the scatter stream (keeps the PE HAM clock
gate at 8/8). A warmup matmul burst upclocks the PE while the first DMAs
stream in; weight/const DMAs and output writes ride the scalar engine's
DMA queue so the sync-queue edge stream is never delayed.
Epilogue per QUAD of 128-atom tiles (512 atoms, feature-major): bf16 MLP
matmuls at N=512, silu on ACT, skip-adds as one fused DVE
scalar_tensor_tensor per layer with host-folded sqrt2 scaling. Output is
written feature-major [P, Cj*T*P] bf16 and untransposed/cast on the host
during unshard.
"""

import math
import os
import sys

import numpy as np
import ml_dtypes

BF16 = ml_dtypes.bfloat16

P = 128
N_CORES = 8
DE, DA, DR, NH = 512, 256, 16, 3
T_ATOM = 20  # atom tiles per core (each up to 128 atoms); divisible by 4
INV_SQRT_2 = 0.7071067811865476

# Of every EVAC_DEN bases-pairs, EVAC_NUM are routed via ACT-evac + 2x DVE;
# the rest are multiplied directly from PSUM at DVE 1x. Balances ACT vs DVE.
EVAC_NUM, EVAC_DEN = 4, 7
PREFETCH = 3  # subtiles of DMA lookahead

_NC_CACHE = {}


# ----------------------------------------------------------------------------
# Host-side packing
# ----------------------------------------------------------------------------

def _pack_layout(idx, n_atoms, n_cores, t_atom):
    E = idx.shape[0]
    n_bins = n_cores * t_atom
    counts = np.bincount(idx, minlength=n_atoms)

    order = np.argsort(-counts, kind="stable")
    n_rounds = math.ceil(n_atoms / n_bins)
    pad = n_rounds * n_bins - n_atoms
    padded = np.concatenate([order, np.full(pad, -1, dtype=order.dtype)])
    grid = padded.reshape(n_rounds, n_bins)
    grid[1::2] = grid[1::2, ::-1]  # snake-deal: balances edges and atoms
    bin_of_atom = np.empty(n_atoms, dtype=np.int64)
    slot_of_atom = np.empty(n_atoms, dtype=np.int64)
    valid = grid >= 0
    bin_idx = np.broadcast_to(np.arange(n_bins), grid.shape)
    round_idx = np.broadcast_to(np.arange(n_rounds)[:, None], grid.shape)
    bin_of_atom[grid[valid]] = bin_idx[valid]
    slot_of_atom[grid[valid]] = round_idx[valid]
    assert np.bincount(bin_of_atom, minlength=n_bins).max() <= P

    ebin = bin_of_atom[idx]
    eslot = slot_of_atom[idx]
    eorder = np.argsort(ebin * (P + 1) + eslot, kind="stable")
    ebin_sorted = ebin[eorder]
    bin_counts = np.bincount(ebin_sorted, minlength=n_bins)
    K = max(1, math.ceil(bin_counts.max() / P))
    bin_starts = np.zeros(n_bins + 1, dtype=np.int64)
    np.cumsum(bin_counts, out=bin_starts[1:])
    pos_in_bin = np.arange(E) - bin_starts[ebin_sorted]

    core_of_bin = np.arange(n_bins) // t_atom
    tile_of_bin = np.arange(n_bins) % t_atom
    return dict(
        K=K,
        eorder=eorder,
        core_of_edge=core_of_bin[ebin_sorted],
        flat_slot=tile_of_bin[ebin_sorted] * (K * P) + pos_in_bin,
        rel_of_edge=eslot[eorder].astype(np.int64),
        bin_of_atom=bin_of_atom,
        slot_of_atom=slot_of_atom,
        core_of_bin=core_of_bin,
        tile_of_bin=tile_of_bin,
    )


def _pack_weights(W_rbf, W_in, res_W1, res_W2):
    Ci, Cj = DE // P, DA // P
    Cr = DA // P
    # wrbf duplicated at partition rows 0:16 and 32:48 so two row-tiled
    # bases matmuls (tile_position (0,0)/(32,0)) share one rhs tile.
    wrbf2 = np.zeros((64, DE), dtype=np.float32)
    wrbf2[0:DR] = W_rbf
    wrbf2[32 : 32 + DR] = W_rbf
    win = W_in.reshape(Ci, P, Cj, P).transpose(1, 0, 2, 3).reshape(P, Ci * Cj * P)
    blocks = []
    c = INV_SQRT_2
    for l in range(NH):
        w1 = (res_W1[l] * (c ** l)).astype(np.float32)
        w2 = res_W2[l].astype(np.float32)
        for W in (w1, w2):
            blocks.append(
                W.reshape(Cr, P, Cr, P).transpose(1, 0, 2, 3).reshape(P, Cr * Cr * P)
            )
    wres = np.concatenate(blocks, axis=1)
    return (
        np.ascontiguousarray(wrbf2, dtype=BF16),
        np.ascontiguousarray(win, dtype=BF16),
        np.ascontiguousarray(wres, dtype=BF16),
    )


def _build_in_maps(m, basis_rad, layout, W_rbf, W_in, res_W1, res_W2, n_cores, t_atom):
    K = layout["K"]
    P_K = (K + 1) // 2
    cap = t_atom * K * P
    ncols = t_atom * K
    eorder = layout["eorder"]
    core_of_edge = layout["core_of_edge"]
    flat_slot = layout["flat_slot"]
    rel = layout["rel_of_edge"]

    wrbf2, win, wres = _pack_weights(W_rbf, W_in, res_W1, res_W2)
    m_src = m[eorder]
    bas_src = basis_rad[eorder]

    in_maps = []
    for c in range(n_cores):
        sel = core_of_edge == c
        fs = flat_slot[sel]
        m_pack = np.zeros((cap, DE), dtype=BF16)
        m_pack[fs] = m_src[sel].astype(BF16)
        # partition-major: m2[p, col*DE + d] = m_pack[col*P + p, d] so each
        # partition's per-atom-tile DMA read is fully contiguous
        m_pack = np.ascontiguousarray(
            m_pack.reshape(ncols, P, DE).transpose(1, 0, 2).reshape(P, ncols, DE)
        )
        basT = np.zeros((DR, cap), dtype=np.float32)
        basT[:, fs] = bas_src[sel].T
        # pair-packed basis, dense 32 rows: rows 0:16 hold even columns,
        # rows 16:32 odd columns; DMA'd as two 16-row slices into SBUF
        # partitions 0:16 and 32:48 for the row-tiled 2-pack bases matmul.
        bt = basT.reshape(DR, t_atom, K, P)
        b2 = np.zeros((32, t_atom, P_K, P), dtype=np.float32)
        b2[0:DR] = bt[:, :, 0::2, :]
        n_odd = K // 2
        if n_odd:
            b2[16 : 16 + DR, :, 0:n_odd, :] = bt[:, :, 1::2, :]
        basT2 = np.ascontiguousarray(
            b2.reshape(32, t_atom * P_K * P), dtype=BF16
        )
        rel_flat = np.full(cap, -1, dtype=np.int64)
        rel_flat[fs] = rel[sel]
        rel2 = rel_flat.reshape(ncols, P).T  # [p, col]
        s_host = (rel2[:, :, None] == np.arange(P)[None, None, :]).astype(
            ml_dtypes.float8_e4m3
        )
        in_maps.append(
            dict(
                m_pack=m_pack,
                basT2=basT2,
                s_hot=np.ascontiguousarray(s_host.reshape(P, ncols * P)),
                wrbf2=wrbf2,
                win=win,
                wres=wres,
            )
        )
    return in_maps


def _unpack_output(results, layout, n_atoms, n_cores, t_atom):
    Cj = DA // P
    out = np.zeros((n_atoms, DA), dtype=np.float32)
    core_of_atom = layout["core_of_bin"][layout["bin_of_atom"]]
    row_of_atom = (
        layout["tile_of_bin"][layout["bin_of_atom"]] * P + layout["slot_of_atom"]
    )
    for c in range(n_cores):
        x = results[c]["out"].astype(np.float32).reshape(P, Cj, t_atom, P)
        x_core = x.transpose(2, 3, 1, 0).reshape(t_atom * P, DA)
        mask = core_of_atom == c
        out[mask] = x_core[row_of_atom[mask]]
    return out


# ----------------------------------------------------------------------------
# Bass kernel builder
# ----------------------------------------------------------------------------

def _build_nc(t_atom, K):
    import concourse.mybir as mybir
    import concourse.tile as tile
    from concourse import bacc

    f32 = mybir.dt.float32
    bf16 = mybir.dt.bfloat16
    Ci, Cj = DE // P, DA // P
    Cr = DA // P
    P_K = (K + 1) // 2
    ncols = t_atom * K
    C3 = INV_SQRT_2 ** NH
    GAMMA = [float((1.0 / INV_SQRT_2) ** l) for l in range(NH)]
    assert t_atom % 4 == 0
    n_quads = t_atom // 4
    W4 = 4 * P  # atoms per epilogue quad

    nc = bacc.Bacc(
        "TRN2",
        target_bir_lowering=False,
        debug=False,
        enable_asserts=False,
        num_devices=N_CORES,
    )
    d_m = nc.dram_tensor("m_pack", [P, ncols, DE], bf16, kind="ExternalInput")
    d_basT2 = nc.dram_tensor(
        "basT2", [32, t_atom * P_K * P], bf16, kind="ExternalInput"
    )
    f8 = mybir.dt.float8e4
    d_s = nc.dram_tensor("s_hot", [P, ncols * P], f8, kind="ExternalInput")
    d_wrbf2 = nc.dram_tensor("wrbf2", [64, DE], bf16, kind="ExternalInput")
    d_win = nc.dram_tensor("win", [P, Ci * Cj * P], bf16, kind="ExternalInput")
    d_wres = nc.dram_tensor(
        "wres", [P, NH * 2 * Cr * Cr * P], bf16, kind="ExternalInput"
    )
    d_out = nc.dram_tensor("out", [P, Cj * t_atom * P], bf16, kind="ExternalOutput")

    with tile.TileContext(nc) as tc:
        with (
            tc.tile_pool(name="const", bufs=1) as const_p,
            tc.tile_pool(name="bas", bufs=6) as bas_p,
            tc.tile_pool(name="m", bufs=6) as m_p,
            tc.tile_pool(name="s", bufs=6) as s_p,
            tc.tile_pool(name="x", bufs=6) as x_p,
            tc.tile_pool(name="bb", bufs=3) as bb_p,
            tc.tile_pool(name="ztsb", bufs=2) as ztsb_p,
            tc.tile_pool(name="act", bufs=3) as act_p,
            tc.tile_pool(name="outp", bufs=2) as out_p,
            tc.tile_pool(name="psb", bufs=2, space="PSUM") as psb_p,
            tc.tile_pool(name="psz", bufs=2, space="PSUM") as psz_p,
            tc.tile_pool(name="psm", bufs=2, space="PSUM") as psm_p,
        ):
            def emit_silu(out_ap, in_ps_ap):
                nc.scalar.activation(
                    out=out_ap, in_=in_ps_ap,
                    func=mybir.ActivationFunctionType.Silu,
                )

            # wrbf2 first on the sync queue so the edge stream follows
            # immediately; the other consts ride the scalar engine's DMA
            # queue so they never delay the edge-stream DMAs.
            wrbf_sb = const_p.tile([64, DE], bf16, tag="wrbf")
            nc.sync.dma_start(out=wrbf_sb[:], in_=d_wrbf2[:])
            win_sb = const_p.tile([P, Ci * Cj * P], bf16, tag="win")
            nc.scalar.dma_start(out=win_sb[:], in_=d_win[:])
            wres_sb = const_p.tile([P, NH * 2 * Cr * Cr * P], bf16, tag="wres")
            nc.scalar.dma_start(out=wres_sb[:], in_=d_wres[:])

            # HAM warmup: dense back-to-back full-K matmuls upclock the PE
            # (4/8 -> 8/8) while the first edge DMAs land. The operand tile
            # is memset on GpSimd (not DMA'd) so the burst starts right
            # after the framework preamble instead of waiting on HBM.
            warm_in = const_p.tile([P, DE], bf16, tag="warmin")
            nc.gpsimd.memset(warm_in[:], 0.0)
            warm_ps = psb_p.tile([P, 2, DE], f32, space="PSUM", tag="bases",
                                 name="warm")
            for w in range(16):
                nc.tensor.matmul(
                    out=warm_ps[:, w % 2, :],
                    lhsT=warm_in[:, (w % 4) * P : (w % 4 + 1) * P],
                    rhs=warm_in[:],
                    start=True,
                    stop=True,
                )

            def issue_sub_dmas(t):
                bas_sb = bas_p.tile([48, P_K * P], bf16, tag="bas",
                                    name=f"bas{t}")
                nc.sync.dma_start(
                    out=bas_sb[0:16, :],
                    in_=d_basT2[0:16, t * P_K * P : (t + 1) * P_K * P],
                )
                nc.sync.dma_start(
                    out=bas_sb[32:48, :],
                    in_=d_basT2[16:32, t * P_K * P : (t + 1) * P_K * P],
                )
                m_t = m_p.tile([P, K, DE], bf16, tag="m", name=f"m{t}")
                s_t = s_p.tile([P, K * P], f8, tag="s", name=f"s{t}")
                if t == 0:
                    # split the very first m transfer (and land s before its
                    # tail) so the leading columns start ~3us earlier
                    h = max(1, K // 3)
                    nc.sync.dma_start(
                        out=m_t[:, 0:h, :], in_=d_m[:, 0:h, :]
                    )
                    nc.sync.dma_start(
                        out=s_t[:], in_=d_s[:, t * K * P : (t + 1) * K * P]
                    )
                    nc.sync.dma_start(
                        out=m_t[:, h:K, :], in_=d_m[:, h:K, :]
                    )
                else:
                    nc.sync.dma_start(
                        out=m_t[:], in_=d_m[:, t * K : (t + 1) * K, :]
                    )
                    nc.sync.dma_start(
                        out=s_t[:], in_=d_s[:, t * K * P : (t + 1) * K * P]
                    )
                return (bas_sb, m_t, s_t)

            def epilogue_gen(q, zt_sb, fill=False):
                """Quad epilogue emitted as units interleavable with the next
                quad's scatter stream (keeps PE continuously busy for HAM).
                With fill=True (the final quad, which drains with no scatter
                stream to interleave into), latency boundaries are padded
                with dummy full-K matmuls so the PE holds its 8/8 clock and
                never sits idle waiting on ACT/DVE chains."""
                fill_t = [None]

                def pad(n):
                    if not fill:
                        return
                    if fill_t[0] is None:
                        fill_t[0] = psb_p.tile(
                            [P, 2, DE], f32, space="PSUM", tag="bases",
                            name="tailfill"
                        )
                    for w in range(n):
                        nc.tensor.matmul(
                            out=fill_t[0][:, 0, :],
                            lhsT=warm_in[:, (w % 4) * P : (w % 4 + 1) * P],
                            rhs=warm_in[:],
                            start=True,
                            stop=True,
                        )

                u_ps = [
                    psm_p.tile(
                        [P, W4], f32, space="PSUM", tag="misc", name=f"ups{q}_{j}"
                    )
                    for j in range(Cj)
                ]
                # sub-major N=128 matmuls with interleaved per-region
                # accumulation: the first stepped units only need subtile 0's
                # evac, so the epilogue never waits on the last subtile.
                for sub in range(4):
                    for c in range(Ci):
                        for j in range(Cj):
                            fi = c * Cj + j
                            nc.tensor.matmul(
                                out=u_ps[j][:, sub * P : (sub + 1) * P],
                                lhsT=win_sb[:, fi * P : (fi + 1) * P],
                                rhs=zt_sb[:, c, sub, :],
                                start=(sub == 0 and c == 0),
                                stop=(sub == 3 and c == Ci - 1),
                                skip_group_check=True,
                            )
                        yield
                X = act_p.tile([P, Cr * W4], bf16, tag="X", name=f"X{q}_0")
                for j in range(Cj):
                    emit_silu(X[:, j * W4 : (j + 1) * W4], u_ps[j][:])
                pad(4)
                yield
                for l in range(NH):
                    v_ps = [
                        psm_p.tile(
                            [P, W4], f32, space="PSUM", tag="misc",
                            name=f"vps{q}_{l}_{j}"
                        )
                        for j in range(Cr)
                    ]
                    for j in range(Cr):
                        for i in range(Cr):
                            fi = ((l * 2 + 0) * Cr + i) * Cr + j
                            nc.tensor.matmul(
                                out=v_ps[j][:],
                                lhsT=wres_sb[:, fi * P : (fi + 1) * P],
                                rhs=X[:, i * W4 : (i + 1) * W4],
                                start=(i == 0),
                                stop=(i == Cr - 1),
                            )
                            yield
                    u1 = act_p.tile([P, Cr * W4], bf16, tag="u1", name=f"u1_{q}_{l}")
                    for j in range(Cr):
                        emit_silu(u1[:, j * W4 : (j + 1) * W4], v_ps[j][:])
                    pad(3)
                    yield
                    w_ps = [
                        psm_p.tile(
                            [P, W4], f32, space="PSUM", tag="misc",
                            name=f"wps{q}_{l}_{j}"
                        )
                        for j in range(Cr)
                    ]
                    for j in range(Cr):
                        for i in range(Cr):
                            fi = ((l * 2 + 1) * Cr + i) * Cr + j
                            nc.tensor.matmul(
                                out=w_ps[j][:],
                                lhsT=wres_sb[:, fi * P : (fi + 1) * P],
                                rhs=u1[:, i * W4 : (i + 1) * W4],
                                start=(i == 0),
                                stop=(i == Cr - 1),
                            )
                            yield
                    Y = act_p.tile([P, Cr * W4], bf16, tag="y", name=f"Y{q}_{l}")
                    for j in range(Cr):
                        emit_silu(Y[:, j * W4 : (j + 1) * W4], w_ps[j][:])
                    pad(3)
                    yield
                    Xn = act_p.tile(
                        [P, Cr * W4], bf16, tag="X", name=f"X{q}_{l + 1}"
                    )
                    nc.vector.scalar_tensor_tensor(
                        out=Xn[:],
                        in0=Y[:],
                        scalar=GAMMA[l],
                        in1=X[:],
                        op0=mybir.AluOpType.mult,
                        op1=mybir.AluOpType.add,
                    )
                    X = Xn
                    pad(4)
                    yield
                o_t = out_p.tile([P, Cj * W4], bf16, tag="out")
                nc.vector.tensor_scalar(
                    out=o_t[:], in0=X[:], scalar1=float(C3), scalar2=None,
                    op0=mybir.AluOpType.mult,
                )
                for j in range(Cj):
                    nc.scalar.dma_start(
                        out=d_out[
                            :, (j * t_atom + 4 * q) * P : (j * t_atom + 4 * q + 4) * P
                        ],
                        in_=o_t[:, j * W4 : (j + 1) * W4],
                    )
                yield

            tiles = {}
            for t in range(min(PREFETCH, t_atom)):
                tiles[t] = issue_sub_dmas(t)

            prev_epi = None
            pair_ctr = [0]
            dmy_state = [None, 0]
            for q in range(n_quads):
                zt_sb = ztsb_p.tile([P, Ci, 4, P], bf16, tag="ztsb")

                def step_epi():
                    if prev_epi is not None:
                        next(prev_epi, None)
                    else:
                        # quad 0 has no epilogue to interleave; burn a full-K
                        # matmul on every other step so HAM sees enough PE
                        # activity to hold the 8/8 clock.
                        w = dmy_state[1]
                        dmy_state[1] += 1
                        if w % 2 == 1:
                            return
                        if dmy_state[0] is None:
                            dmy_state[0] = psm_p.tile(
                                [P, W4], f32, space="PSUM", tag="misc",
                                name="dmy"
                            )
                        nc.tensor.matmul(
                            out=dmy_state[0][:],
                            lhsT=warm_in[:, (w % 4) * P : (w % 4 + 1) * P],
                            rhs=warm_in[:],
                            start=True,
                            stop=True,
                        )

                for sub in range(4):
                    t = 4 * q + sub
                    tf = t + PREFETCH
                    if tf < t_atom:
                        tiles[tf] = issue_sub_dmas(tf)
                    bas_sb, m_t, s_t = tiles.pop(t)
                    zt_ps = psz_p.tile(
                        [P, Ci, P], f32, space="PSUM", tag="z", name=f"ztps{t}"
                    )

                    def emit_mult(psb_t, p, evac, m_t=m_t, t=t):
                        """DVE/ACT multiply, emitted right at pair issue so the
                        mult chain runs several pairs ahead of the PE's
                        scatter consumption."""
                        c0 = 2 * p
                        w = 2 if c0 + 1 < K else 1
                        x_t = x_p.tile([P, 2, DE], bf16, tag="x",
                                       name=f"x{t}_{p}")
                        if evac:
                            bb = bb_p.tile([P, 2, DE], bf16, tag="bb",
                                           name=f"bb{t}_{p}")
                            nc.scalar.copy(
                                out=bb[:, 0:w, :], in_=psb_t[:, 0:w, :]
                            )
                            nc.vector.tensor_tensor(
                                out=x_t[:, 0:w, :],
                                in0=bb[:, 0:w, :],
                                in1=m_t[:, c0 : c0 + w, :],
                                op=mybir.AluOpType.mult,
                            )
                        else:
                            nc.vector.tensor_tensor(
                                out=x_t[:, 0:w, :],
                                in0=psb_t[:, 0:w, :],
                                in1=m_t[:, c0 : c0 + w, :],
                                op=mybir.AluOpType.mult,
                            )
                        return x_t

                    def emit_scatter(item, s_t=s_t, zt_ps=zt_ps):
                        x_t, p = item
                        c0 = 2 * p
                        w = 2 if c0 + 1 < K else 1
                        for i in range(w):
                            c = c0 + i
                            for ci in range(Ci):
                                nc.tensor.matmul(
                                    out=zt_ps[:, ci, :],
                                    lhsT=x_t[:, i, ci * P : (ci + 1) * P],
                                    rhs=s_t[:, c * P : (c + 1) * P],
                                    start=(c == 0 and ci == 0),
                                    stop=(c == K - 1 and ci == Ci - 1),
                                    skip_group_check=True,
                                )

                    # pairs are emitted in blocks of two: consecutive pairs
                    # reuse the same row-groups back-to-back, so the ~100ns
                    # row-tile -> full-row handoff is paid once per block
                    # instead of once per pair.
                    pend = []
                    for pb in range(0, P_K, 2):
                        blk = [p for p in (pb, pb + 1) if p < P_K]
                        xs = []
                        for p in blk:
                            c0 = 2 * p
                            psb_t = psb_p.tile(
                                [P, 2, DE], f32, space="PSUM", tag="bases",
                                name=f"bps{t}_{p}"
                            )
                            nc.tensor.matmul(
                                out=psb_t[:, 0, :],
                                lhsT=bas_sb[0:DR, p * P : (p + 1) * P],
                                rhs=wrbf_sb[0:DR, :],
                                start=True,
                                stop=True,
                                tile_position=(0, 0),
                            )
                            if c0 + 1 < K:
                                nc.tensor.matmul(
                                    out=psb_t[:, 1, :],
                                    lhsT=bas_sb[32 : 32 + DR, p * P : (p + 1) * P],
                                    rhs=wrbf_sb[32 : 32 + DR, :],
                                    start=True,
                                    stop=True,
                                    tile_position=(32, 0),
                                )
                            xs.append((psb_t, p))
                        for psb_t, p in xs:
                            evac = (pair_ctr[0] * EVAC_NUM) % EVAC_DEN < EVAC_NUM
                            pair_ctr[0] += 1
                            pend.append((emit_mult(psb_t, p, evac), p))
                        # epilogue steps between the bases pair and the
                        # scatter burst: a full-row N=512 epilogue matmul
                        # absorbs the row-tile -> full-row drain handoff that
                        # a short N=128 scatter matmul would otherwise pay.
                        for _ in range(2 * len(blk)):
                            step_epi()
                        while len(pend) > 2:
                            emit_scatter(pend.pop(0))
                    for item in pend:
                        emit_scatter(item)

                    # single fused evac: zT chunks -> bf16 zt_sb, no PE work
                    nc.scalar.copy(out=zt_sb[:, :, sub, :], in_=zt_ps[:])
                    step_epi()

                if prev_epi is not None:
                    for _ in prev_epi:
                        pass
                prev_epi = epilogue_gen(q, zt_sb, fill=(q == n_quads - 1))
            for _ in prev_epi:
                pass

    nc.compile()
    return nc


def _get_nc(t_atom, K):
    key = (t_atom, K)
    if key not in _NC_CACHE:
        _NC_CACHE[key] = _build_nc(t_atom, K)
    return _NC_CACHE[key]


# ----------------------------------------------------------------------------
# Entry point
# ----------------------------------------------------------------------------

def kernel(h, m, basis_rad, idx_atom, W_rbf, W_in, res_W1, res_W2):
    from concourse.bass_utils import run_bass_kernel_spmd

    m = np.asarray(m, dtype=np.float32)
    basis_rad = np.asarray(basis_rad, dtype=np.float32)
    idx = np.asarray(idx_atom).astype(np.int64)
    W_rbf = np.asarray(W_rbf, dtype=np.float32)
    W_in = np.asarray(W_in, dtype=np.float32)
    res_W1 = np.asarray(res_W1, dtype=np.float32)
    res_W2 = np.asarray(res_W2, dtype=np.float32)
    n_atoms = np.asarray(h).shape[0]

    layout = _pack_layout(idx, n_atoms, N_CORES, T_ATOM)
    in_maps = _build_in_maps(
        m, basis_rad, layout, W_rbf, W_in, res_W1, res_W2, N_CORES, T_ATOM
    )
    nc = _get_nc(T_ATOM, layout["K"])

    trace = os.environ.get("KERNEL_TRACE", "0") == "1"
    res = run_bass_kernel_spmd(
        nc, in_maps, core_ids=list(range(N_CORES)), trace=trace
    )
    if trace and res.exec_time_ns is not None:
        print(f"HW exec time: {res.exec_time_ns} ns", file=sys.stderr)
        kernel.last_exec_time_ns = res.exec_time_ns
    kernel.last_results = res
    return _unpack_output(res.results, layout, n_atoms, N_CORES, T_ATOM)
